# revision 1
# baseline (speedup 1.0000x reference)
"""Trainium2 Bass kernel for nn_MultiHeadAttention_53017076301867.

Strategy (8 cores, tensor-parallel over H=16 heads, 2 heads/core):
  - Host pre-shards: per-core QKV weight column slices, W_proj row slices,
    x transposed to [E, S] bf16 (layout prep), plus mask-derived rows.
  - Each core computes its 2 heads' global causal attention + the (tiny,
    restructured) local windowed branch + a partial output projection
    over its 128 ctx features.
  - Host sums the 8 partial projections (the "all-reduce") + b_proj.

Math restructure of the reference local branch (validated to 5e-7 rel):
  - chunk-mean-then-mask == mask(mean); exp(-1.25e8) == 0 -> zero masked.
  - softmax rows q<WIN see 256 real scores + (S-WIN) zero-scores
    -> Z = sum(expS) + (S-WIN); numerator += sum_{k>=WIN} vloc[k].
  - rows q>=WIN are uniform -> mean of vloc rows.
All attention is computed in transposed form ctx^T[feat, q] so the final
projection (contract over feat) needs no transposes; per-q normalization
and global/local mask blending are folded into rank-1 PE broadcasts.
"""

import numpy as np
import ml_dtypes

S, E, H, WIN, D = 2048, 1024, 16, 256, 64
C = S // WIN            # 8 chunks
NCORES = 8
SCALE = 1.0 / (D ** 0.5)  # 0.125
BF = ml_dtypes.bfloat16

_prog_cache = {}


def build_program():
    from contextlib import ExitStack
    import concourse.tile as tile
    import concourse.mybir as mybir
    from concourse import bacc
    from concourse.masks import make_identity

    dt = mybir.dt
    f32, bf = dt.float32, dt.bfloat16
    AF = mybir.ActivationFunctionType
    ALU = mybir.AluOpType

    nc = bacc.Bacc("TRN2", target_bir_lowering=False, debug=False)

    xT = nc.dram_tensor("xT", [E, S], bf, kind="ExternalInput").ap()
    xTl = nc.dram_tensor("xTl", [E, 2 * 128], bf, kind="ExternalInput").ap()
    wqkv = nc.dram_tensor("wqkv", [E, 3, 128], bf, kind="ExternalInput").ap()
    wloc = nc.dram_tensor("wloc", [E, 16, 192], bf, kind="ExternalInput").ap()
    lmask = nc.dram_tensor("lmask", [2, 128, WIN], bf, kind="ExternalInput").ap()
    wpr = nc.dram_tensor("wpr", [128, E], bf, kind="ExternalInput").ap()
    mrow = nc.dram_tensor("mrow", [1, S], f32, kind="ExternalInput").ap()
    wcr = nc.dram_tensor("wcr", [1, S], f32, kind="ExternalInput").ap()
    wbr = nc.dram_tensor("wbr", [1, WIN], f32, kind="ExternalInput").ap()
    outp = nc.dram_tensor("outp", [S, E], f32, kind="ExternalOutput").ap()

    with tile.TileContext(nc) as tc, ExitStack() as ctx:
        P = ctx.enter_context(tc.tile_pool(name="persist", bufs=1))

        # ---- input loads: phase-1 operands first, wloc (phase-2) last ----
        wqkv_sb = P.tile([128, 8, 3, 128], bf)
        nc.sync.dma_start(out=wqkv_sb, in_=wqkv.rearrange("(c p) t d -> p c t d", p=128))
        xT_sb = P.tile([128, 8, S], bf)
        xTv = xT.rearrange("(c p) s -> p c s", p=128)
        for ec in range(8):
            nc.sync.dma_start(out=xT_sb[:, ec, :], in_=xTv[:, ec, :])
        xTl_sb = P.tile([128, 8, 256], bf)
        nc.scalar.dma_start(out=xTl_sb, in_=xTl.rearrange("(c p) s -> p c s", p=128))
        mrow_sb = P.tile([1, S], f32)
        nc.sync.dma_start(out=mrow_sb, in_=mrow)
        wcr_sb = P.tile([1, S], f32)
        nc.sync.dma_start(out=wcr_sb, in_=wcr)
        wbr_sb = P.tile([1, WIN], f32)
        nc.sync.dma_start(out=wbr_sb, in_=wbr)
        wpr_sb = P.tile([128, E], bf)
        nc.scalar.dma_start(out=wpr_sb, in_=wpr)
        wloc_sb = P.tile([128, 8, 16, 192], bf)
        wlv = wloc.rearrange("(c p) i v -> p c i v", p=128)
        for half in range(2):
            nc.sync.dma_start(out=wloc_sb[:, :, half * 8:(half + 1) * 8, :],
                              in_=wlv[:, :, half * 8:(half + 1) * 8, :])
        lmask_sb = P.tile([128, 2, WIN], bf)
        nc.sync.dma_start(out=lmask_sb, in_=lmask.rearrange("k p w -> p k w"))

        ones64f = P.tile([1, 64], f32)
        nc.vector.memset(ones64f, 1.0)
        identb128 = P.tile([128, 128], bf)
        make_identity(nc, identb128)
        identf128 = P.tile([128, 128], f32)
        make_identity(nc, identf128)
        onesrow = P.tile([128, WIN], bf)
        nc.vector.memset(onesrow, 0.0)
        nc.vector.memset(onesrow[0:1, :], 1.0)

        QT2 = P.tile([128, S], bf)       # Q^T, rows = 2 heads x 64 dims
        KT2 = P.tile([128, S], bf)
        V2e = P.tile([128, 16, 2, 65], bf)  # [k-part, k-tile, head, d|ones]
        QP = P.tile([64, 2, 8, 16, 16], bf)  # [d, head, u, i, j], perm w~=i*16+j
        KP = P.tile([64, 2, 8, 16, 16], bf)
        VP = P.tile([128, 2, 8, 16, 16], bf)  # rows 64:128 (from paired matmul)
        vloc256e = P.tile([128, 2, 2, 65], bf)
        tail_lhs = P.tile([128, 2, 65], bf)
        vbar = P.tile([1, 2, 64], f32)
        bloc = P.tile([128, WIN], f32)
        ctxT = P.tile([128, S], bf)

        # ---------------- phase 1: global QKV projections ----------------
        with tc.tile_pool(name="ps1", bufs=1, space="PSUM") as ps1, \
                tc.tile_pool(name="sb1", bufs=1) as sb1:
            VT2 = sb1.tile([128, S], bf)
            for tsel, dest in ((0, QT2), (1, KT2), (2, VT2)):
                pss = [ps1.tile([128, 512], f32, tag=f"qk{g}", bufs=1,
                                name=f"qkps{g}") for g in range(4)]
                for ec in range(8):
                    for g in range(4):
                        nc.tensor.matmul(
                            pss[g], lhsT=wqkv_sb[:, ec, tsel, :],
                            rhs=xT_sb[:, ec, g * 512:(g + 1) * 512],
                            start=(ec == 0), stop=(ec == 7),
                            skip_group_check=True)
                for g in range(4):
                    nc.scalar.copy(dest[:, g * 512:(g + 1) * 512], pss[g])
            for st in range(16):
                pv = ps1.tile([128, 128], bf, tag="vtr", bufs=2)
                nc.tensor.transpose(
                    pv, VT2[:, st * 128:(st + 1) * 128], identb128)
                nc.vector.tensor_copy(
                    V2e[:, st, :, 0:64], pv.rearrange("p (h d) -> p h d", h=2))
            nc.vector.memset(V2e[:, :, :, 64], 1.0)

        # ------- phase 2+3 fused: local-proj units woven into global attn ----
        with tc.tile_pool(name="ps3", bufs=2, space="PSUM") as ps3, \
                tc.tile_pool(name="sb3", bufs=4) as sb3:

            # -- local-branch emission helpers (each emits a chunk of work) --
            units = [("qv", i) for i in range(16)] + \
                    [("k", i) for i in range(16)]
            uidx = [0]

            def emit_unit():
                if uidx[0] >= len(units):
                    return
                kind, i = units[uidx[0]]
                uidx[0] += 1
                dve = bool(uidx[0] % 2)
                if kind == "qv":
                    ps = ps3.tile([128, 256], f32, tag="aux", bufs=2,
                                  name="lqvps")
                    for ec in range(8):
                        nc.tensor.matmul(
                            ps, lhsT=wloc_sb[:, ec, i, 0:128],
                            rhs=xTl_sb[:, ec, :],
                            start=(ec == 0), stop=(ec == 7))
                    qsrc = ps[0:64, :].rearrange("d (h u j) -> d h u j", h=2, u=8)
                    vsrc = ps[64:128, :].rearrange("d (h u j) -> d h u j", h=2, u=8)
                    qdst = QP[:, :, :, i, :]
                    vdst = VP[64:128, :, :, i, :]
                    if dve:
                        nc.vector.tensor_copy(qdst, qsrc)
                        nc.scalar.copy(vdst, vsrc)
                    else:
                        nc.scalar.copy(qdst, qsrc)
                        nc.vector.tensor_copy(vdst, vsrc)
                else:
                    ps = ps3.tile([64, 256], f32, tag="aux", bufs=2,
                                  name="lkps")
                    for ec in range(8):
                        nc.tensor.matmul(
                            ps, lhsT=wloc_sb[:, ec, i, 128:192],
                            rhs=xTl_sb[:, ec, :],
                            start=(ec == 0), stop=(ec == 7))
                    src_ = ps.rearrange("d (h u j) -> d h u j", h=2, u=8)
                    if dve:
                        nc.vector.tensor_copy(KP[:, :, :, i, :], src_)
                    else:
                        nc.scalar.copy(KP[:, :, :, i, :], src_)

            def emit_vblock():
                # tail/all sums of vloc rows + vloc256e (needs VP complete)
                ihi_b = identb128[64:128, 64:128]
                ihi_f = identf128[64:128, 64:128]
                for hh in range(2):
                    tcol = sb3.tile([128, 1], f32, tag="tcol")
                    nc.vector.reduce_sum(tcol[64:128, :], VP[64:128, hh, 1:8, :, :],
                                         axis=mybir.AxisListType.XYZ)
                    vallc = sb3.tile([128, 1], f32, tag="vallc")
                    nc.vector.reduce_sum(vallc[64:128, :], VP[64:128, hh, :, :, :],
                                         axis=mybir.AxisListType.XYZ)
                    tcolb = sb3.tile([128, 1], bf, tag="tcolb")
                    nc.vector.tensor_copy(tcolb[64:128, :], tcol[64:128, :])
                    pst = ps3.tile([1, 64], bf, tag="aux", bufs=2, name="trow")
                    nc.tensor.transpose(pst, tcolb[64:128, :], ihi_b)
                    nc.vector.memset(tail_lhs[:, hh, :], 0.0)
                    nc.vector.tensor_copy(tail_lhs[0:1, hh, 0:64], pst)
                    nc.vector.memset(tail_lhs[0:1, hh, 64:65], float(S - WIN))
                    psv = ps3.tile([1, 64], f32, tag="aux", bufs=2, name="vrow")
                    nc.tensor.transpose(psv, vallc[64:128, :], ihi_f)
                    nc.vector.tensor_copy(vbar[:, hh, :], psv)
                    for kt in range(2):
                        pst2 = ps3.tile([128, 64], bf, tag="aux", bufs=2,
                                        name="vtrow")
                        nc.tensor.transpose(
                            pst2, VP[64:128, hh, 0, kt * 8:(kt + 1) * 8, :],
                            ihi_b)
                        nc.vector.tensor_copy(vloc256e[:, hh, kt, 0:64], pst2)
                nc.vector.memset(vloc256e[:, :, :, 64], 1.0)

            def emit_locattn():
                # local windowed attention + B-term (needs QP/KP/vloc256e)
                for hh in range(2):
                    ploc = ps3.tile([65, WIN], f32, tag="aux", bufs=2, name="plocps")
                    for kt in range(2):
                        sps = ps3.tile([128, WIN], f32, tag="aux", bufs=2,
                                       name="slocps")
                        for u in range(8):
                            nc.tensor.matmul(
                                sps, lhsT=KP[:, hh, u, kt * 8:(kt + 1) * 8, :],
                                rhs=QP[:, hh, u, :, :],
                                start=(u == 0), stop=(u == 7))
                        et = sb3.tile([128, WIN], bf, tag="eloc")
                        nc.scalar.activation(et, sps, AF.Exp, scale=SCALE / C)
                        nc.vector.tensor_mul(et, et, lmask_sb[:, kt, :])
                        nc.tensor.matmul(ploc, lhsT=vloc256e[:, hh, kt, :],
                                         rhs=et, start=(kt == 0), stop=False,
                                         skip_group_check=True)
                    nc.tensor.matmul(ploc, lhsT=tail_lhs[:, hh, :],
                                     rhs=onesrow, start=False, stop=True,
                                     skip_group_check=True)
                    zl = sb3.tile([1, WIN], f32, tag="zl")
                    nc.vector.reciprocal(zl, ploc[64:65, :])
                    rbl = sb3.tile([1, WIN], f32, tag="rbl")
                    nc.vector.tensor_mul(rbl, zl, wbr_sb)
                    rblp = ps3.tile([64, WIN], f32, tag="aux", bufs=2,
                                    name="rblps")
                    nc.tensor.matmul(rblp, lhsT=ones64f, rhs=rbl,
                                     start=True, stop=True)
                    rbls = sb3.tile([64, WIN], f32, tag="rbls")
                    nc.vector.tensor_copy(rbls, rblp)
                    nc.vector.tensor_mul(bloc[hh * 64:(hh + 1) * 64, :],
                                         ploc[0:64, :], rbls)

            # -- global causal attention, software-pipelined --
            NSTEP = sum(4 * g + 4 for g in range(4))       # 40
            drain = [0]

            def pace(step_no):
                # qv units (VP dep) by step 16; spread k units to step 40
                if step_no < 16:
                    want = step_no + 1
                else:
                    want = 16 + ((step_no - 15) * 16 + 23) // 24
                want = min(len(units), want)
                while uidx[0] < want:
                    emit_unit()

            pjn = [0]

            def emit_proj(qt):
                for half in range(2):
                    pp = ps3.tile([128, 512], f32, tag="aux", bufs=2,
                                  name="ppps")
                    nc.tensor.matmul(
                        pp, lhsT=ctxT[:, qt * 128:(qt + 1) * 128],
                        rhs=wpr_sb[:, half * 512:(half + 1) * 512],
                        start=True, stop=True)
                    ot = sb3.tile([128, 512], f32, tag="ot", bufs=3)
                    pjn[0] += 1
                    if pjn[0] % 2:
                        nc.vector.tensor_copy(ot, pp)
                    else:
                        nc.scalar.copy(ot, pp)
                    nc.sync.dma_start(
                        out=outp[qt * 128:(qt + 1) * 128,
                                 half * 512:(half + 1) * 512],
                        in_=ot)

            step = [0]
            deferred = []

            def run_deferred():
                for fn in deferred:
                    fn()
                deferred.clear()

            def make_blend(g, gpss):
                def blend():
                    for hh in range(2):
                        hs = slice(hh * 64, hh * 64 + 64)
                        gsb = gpss[hh]
                        zrow = sb3.tile([1, 512], f32, tag="zrow")
                        nc.vector.reciprocal(zrow, gsb[64:65, :])
                        ra = sb3.tile([1, 512], f32, tag="ra")
                        nc.vector.tensor_mul(
                            ra, zrow, mrow_sb[:, g * 512:(g + 1) * 512])
                        rbp = ps3.tile([64, 512], f32, tag="aux", bufs=2,
                                       name="rbps")
                        nc.tensor.matmul(rbp, lhsT=ones64f, rhs=ra,
                                         start=True, stop=True)
                        rbs = sb3.tile([64, 512], f32, tag="rbs", bufs=2)
                        nc.vector.tensor_copy(rbs, rbp)
                        cps = ps3.tile([64, 512], f32, tag="aux", bufs=2,
                                       name="cpps")
                        nc.tensor.matmul(
                            cps, lhsT=vbar[:, hh, :],
                            rhs=wcr_sb[:, g * 512:(g + 1) * 512],
                            start=True, stop=True)
                        dst = ctxT[hs, g * 512:(g + 1) * 512]
                        nc.vector.tensor_mul(dst, gsb[0:64, :], rbs)
                        nc.vector.tensor_add(dst, dst, cps)
                    for qt in range(4 * g + 3, 4 * g - 1, -1):
                        if qt >= 2:
                            emit_proj(qt)
                return blend

            # descending g: by the end of g=3's 16 steps all VP units have
            # drained (vblock dep), g=1 end covers QP/KP (locattn dep)
            for g in (3, 2, 1, 0):
                gpss = [ps3.tile([65, 512], f32, tag=f"g{h}", bufs=1,
                                 name=f"gctxps{h}") for h in range(2)]
                nkt = 4 * g + 4
                pend = []
                for t in range(nkt + 2):
                    if t < nkt:
                        # boundary tiles: only q-columns >= k are live
                        off = (t % 4) * 128 if t // 4 == g else 0
                        sps = ps3.tile([128, 2, 512], f32, tag="sT", bufs=2,
                                       name="sTps")
                        for hh in range(2):
                            hs = slice(hh * 64, hh * 64 + 64)
                            nc.tensor.matmul(
                                sps[:, hh, off:],
                                lhsT=KT2[hs, t * 128:(t + 1) * 128],
                                rhs=QT2[hs, g * 512 + off:(g + 1) * 512],
                                start=True, stop=True, skip_group_check=True)
                        et = sb3.tile([128, 2, 512], bf, tag="expT", bufs=4)
                        nc.scalar.activation(et[:, :, off:], sps[:, :, off:],
                                             AF.Exp, scale=SCALE)
                        if t >= 4 * g:
                            nc.gpsimd.affine_select(
                                et[:, :, off:], et[:, :, off:],
                                pattern=[[0, 2], [1, 512 - off]],
                                base=0, channel_multiplier=-1,
                                compare_op=ALU.is_ge, fill=0.0)
                        pend.append((t, et, off))
                        pace(step[0])
                        step[0] += 1
                    if len(pend) > 2 or (t >= nkt and pend):
                        pt, pet, poff = pend.pop(0)
                        for hh in range(2):
                            nc.tensor.matmul(
                                gpss[hh][:, poff:], lhsT=V2e[:, pt, hh, :],
                                rhs=pet[:, hh, poff:],
                                start=(pt == 0), stop=(pt == nkt - 1),
                                skip_group_check=True)
                if g == 3:
                    emit_vblock()
                if g == 1:
                    while uidx[0] < len(units):
                        emit_unit()
                    emit_locattn()
                make_blend(g, gpss)()
            for hh in range(2):
                hs = slice(hh * 64, hh * 64 + 64)
                dperm = ctxT[hs, 0:WIN].rearrange("p (j i) -> p i j", j=16)
                nc.vector.tensor_add(dperm, dperm, bloc[hs, :])
            for qt in (1, 0):
                emit_proj(qt)

    nc.compile()
    return nc

def prep_inputs(x, global_attention_mask, W_local_query, W_local_key,
                W_local_value, W_query, W_key, W_value, W_proj):
    """Host-side sharding/layout prep. Returns list of per-core input dicts."""
    def b(a):
        return np.ascontiguousarray(np.asarray(a, np.float32)).astype(BF)

    x2 = np.asarray(x, np.float32).reshape(S, E)
    xT_np = np.ascontiguousarray(x2.T).astype(BF)                   # [E, S]
    # per-i interleave [q_i | v_i | k_i] for paired local projections
    Wlq = np.asarray(W_local_query, np.float32).reshape(E, 16, 64)
    Wlk = np.asarray(W_local_key, np.float32).reshape(E, 16, 64)
    Wlv = np.asarray(W_local_value, np.float32).reshape(E, 16, 64)
    wloc_np = np.ascontiguousarray(
        np.concatenate([Wlq, Wlv, Wlk], axis=2)).astype(BF)  # [E, 16, 192]
    # local causal mask in permuted order w~ = i*16 + j (true w = j*16 + i)
    wt = np.arange(WIN)
    w_of = (wt % 16) * 16 + wt // 16
    lmask_np = np.ascontiguousarray(
        (w_of.reshape(2, 128)[:, :, None] <= w_of[None, None, :])
        .astype(np.float32)).astype(BF)                       # [2, 128, WIN]
    m = np.asarray(global_attention_mask, np.float32).reshape(S)
    q = np.arange(S)
    mrow_np = np.ascontiguousarray(m.reshape(1, S))
    wcr_np = np.ascontiguousarray(((1.0 - m) * (q >= WIN) / S).reshape(1, S)
                                  ).astype(np.float32)
    wbr_np = np.ascontiguousarray(((1.0 - m)[w_of]).reshape(1, WIN)
                                  ).astype(np.float32)        # permuted order
    Wq = np.asarray(W_query, np.float32)
    Wk = np.asarray(W_key, np.float32)
    Wv = np.asarray(W_value, np.float32)
    Wp = np.asarray(W_proj, np.float32)

    in_maps = []
    for i in range(NCORES):
        cs = slice(i * 128, (i + 1) * 128)
        in_maps.append({
            "xT": xT_np,
            "xTl": np.ascontiguousarray(xT_np[:, i * 256:(i + 1) * 256]),
            "wqkv": b(np.stack([Wq[:, cs], Wk[:, cs], Wv[:, cs]], axis=1)),
            "wloc": wloc_np,
            "lmask": lmask_np,
            "wpr": b(Wp[cs, :]),
            "mrow": mrow_np,
            "wcr": wcr_np,
            "wbr": wbr_np,
        })
    return in_maps


def kernel(x, global_attention_mask, W_local_query, W_local_key, W_local_value,
           W_query, W_key, W_value, W_proj, b_proj):
    from concourse.bass_utils import run_bass_kernel_spmd

    if "nc" not in _prog_cache:
        _prog_cache["nc"] = build_program()
    nc = _prog_cache["nc"]

    in_maps = prep_inputs(x, global_attention_mask, W_local_query, W_local_key,
                          W_local_value, W_query, W_key, W_value, W_proj)
    res = run_bass_kernel_spmd(nc, in_maps, core_ids=list(range(NCORES)))
    out = np.zeros((S, E), np.float32)
    for r in res.results:
        out += r["outp"]
    out = out + np.asarray(b_proj, np.float32)[None, :]
    return out[None].astype(np.float32)

